# revision 25
# baseline (speedup 1.0000x reference)
"""Trainium2 Bass kernel for causal multi-head attention with RoPE.

Problem: x[2,2048,2048], 16 heads, head_dim 128, fp32.
  q/k/v = x @ w{q,k,v}^T ; RoPE on q,k ; causal softmax(q k^T / sqrt(128)) @ v ; out @ wo^T

Sharding: Megatron tensor-parallel over heads - 2 heads per core on 8 cores.
Each core computes a partial y (its 2 heads' contribution through wo); the host
sums the 8 partials.  No device collectives.

Schedule (v2, restructured from the 395us baseline after trace analysis showed
PE stalling ~1us per kt-pair on the ACT exp latency inside attention units):
  - each half-tile slot tt = [phase A: projection c-loop x16 with unit tt-1's
    score matmuls+exp interleaved between c-iterations] then [phase B: drains,
    yproj(tt-2), AV(tt-1), normalize, RoPE(tt)].  Scores are emitted as late
    as possible in phase A (offset CCH - nslots) while still giving every exp
    a full c-iteration of PE work to hide under; by AV time all exps are done,
    so phase B's AV+yproj matmuls run back-to-back.
  - probs pt live in SBUF (16 tiles), decoupling exp from AV; PSUM is
    statically 8 banks: qk(2x2, reused by yproj py), pv(2x1, reused by AV po),
    pp(1x2 for in-flight scores).
  - softmax denominators: DVE pair-sum accumulation into f16 acc as before,
    then ONE gpsimd partition_all_reduce per head (replaces the rowsum
    matmuls + PE transposes + ACT copy + broadcast chain), DVE reciprocal
    [128,512] -> bf16, one DVE mul onorm = po * rinv.
  - causal masking: single [128,128] triangular mask applied to the diagonal
    128-col band only (cols below the band are excluded from exp/AV/acc by
    offsets); exp split on the second diagonal pair to skip fully-masked cols.
  - drains: q/k PSUM->SBUF casts on gpsimd (idle engine), v+ystage upper on
    ACT, ystage lower on DVE.
  - startup: x chunk0 + weight group-0 DMAs 4-way partition-split round-robin
    over 4 queues (sync/gpsimd/scalar/vector) before any const emission, so
    the first c-loop starts ~8us earlier.
"""

import math
import sys

sys.path.insert(0, "/opt/trn_rl_repo")

import ml_dtypes  # noqa: E402
import numpy as np  # noqa: E402

P = 128
D = 2048
HD = 128  # head dim
B = 2
T = 2048
TOK = B * T  # 4096
NCORES = 8
HPC = 2  # heads per core
DC = HPC * HD  # 256 dims per core
CCH = D // P  # 16 contraction chunks
NPAIR = TOK // 1024  # 4 token-tile pairs (1024 tokens each)
QT = T // 512  # 4 query tiles per batch

_CACHE = {}


def _build_nc():
    import concourse.bacc as bacc

    import concourse.mybir as mybir
    import concourse.tile as tile

    f32 = mybir.dt.float32
    bf16 = mybir.dt.bfloat16
    f16 = mybir.dt.float16
    Exp = mybir.ActivationFunctionType.Exp

    nc = bacc.Bacc("TRN2", target_bir_lowering=False, debug=False, num_devices=NCORES)

    xb = nc.dram_tensor("xb", [NPAIR, CCH, P, 2, 512], bf16, kind="ExternalInput").ap()
    csb = nc.dram_tensor("csb", [NPAIR, P, 2, 512], bf16, kind="ExternalInput").ap()
    snb = nc.dram_tensor("snb", [NPAIR, P, 2, 512], bf16, kind="ExternalInput").ap()
    wqb = nc.dram_tensor("wqb", [P, 4, 1024], bf16, kind="ExternalInput").ap()
    wkb = nc.dram_tensor("wkb", [P, 4, 1024], bf16, kind="ExternalInput").ap()
    wvb = nc.dram_tensor("wvb", [P, 4, 1024], bf16, kind="ExternalInput").ap()
    wob = nc.dram_tensor("wob", [P, HPC, D], bf16, kind="ExternalInput").ap()
    y = nc.dram_tensor("y", [TOK, 4, 512], f16, kind="ExternalOutput").ap()

    inv_sqrt_hd = 1.0 / math.sqrt(HD)

    with tile.TileContext(nc) as tc:
        with (
            tc.tile_pool(name="consts", bufs=1) as consts,
            tc.tile_pool(name="wpool", bufs=1) as wpool,
            tc.tile_pool(name="qkv", bufs=1) as qkv,
            tc.tile_pool(name="xp", bufs=17) as xp,
            tc.tile_pool(name="csp", bufs=2) as csp,
            tc.tile_pool(name="ropep", bufs=2) as ropep,
            tc.tile_pool(name="ptp", bufs=16) as ptp,
            tc.tile_pool(name="accp", bufs=4) as accp,
            tc.tile_pool(name="rbcp", bufs=2) as rbcp,
            tc.tile_pool(name="rinvp", bufs=2) as rinvp,
            tc.tile_pool(name="onp", bufs=3) as onp,
            tc.tile_pool(name="ysp", bufs=3) as ysp,
            tc.tile_pool(name="ps", bufs=1, space="PSUM") as ps,
        ):
            # ---- resident weights (SBUF-image dram layouts: 2KB+ rows) ----
            wq_t = wpool.tile([P, 4, 1024], bf16, tag="wq", name="wq_t")
            wk_t = wpool.tile([P, 4, 1024], bf16, tag="wk", name="wk_t")
            wv_t = wpool.tile([P, 4, 1024], bf16, tag="wv", name="wv_t")
            wo_t = wpool.tile([P, HPC, D], bf16, tag="wo", name="wo_t")

            # ---- resident activations ----
            qT_t = qkv.tile([P, HPC, TOK], bf16, tag="qT", name="qT_t")
            kT_t = qkv.tile([P, HPC, TOK], bf16, tag="kT", name="kT_t")
            v_t = qkv.tile([P, TOK // P, DC], bf16, tag="v", name="v_t")

            QUEUES3 = [nc.sync, nc.gpsimd, nc.scalar]

            # ---- startup burst (baseline pattern): weight group-0 4-way
            # partition-split across the 3 DMA queues, before anything else.
            for wi, (wt, wd) in enumerate(((wq_t, wqb), (wk_t, wkb), (wv_t, wvb))):
                for s4 in range(4):
                    psl = slice(s4 * 32, (s4 + 1) * 32)
                    QUEUES3[(wi + s4) % 3].dma_start(wt[psl, 0, :], wd[psl, 0, :])

            # ---- constants (tiny; tri not needed until slot 1's scores) ----
            # triangular 0/1 mask for the diagonal 128-col band: keep c >= p
            tri = consts.tile([P, P], bf16, tag="tri", name="tri")
            ones_col = consts.tile([P, 1], f16, tag="ones_col", name="ones_col")
            ones_rows = consts.tile([64, P], bf16, tag="ones_rows", name="ones_rows")

            def emit_consts():
                nc.gpsimd.memset(tri[:], 1.0)
                nc.gpsimd.affine_select(
                    out=tri[:], in_=tri[:], compare_op=mybir.AluOpType.is_ge,
                    fill=0.0, base=0, channel_multiplier=-1, pattern=[[1, P]],
                )
                nc.gpsimd.memset(ones_col[:], 1.0)
                nc.gpsimd.memset(ones_rows[:], 1.0)

            def emit_w_group(g, nsplit):
                for wi, (wt, wd) in enumerate(((wq_t, wqb), (wk_t, wkb),
                                               (wv_t, wvb))):
                    for s in range(nsplit):
                        psl = slice(s * (P // nsplit), (s + 1) * (P // nsplit))
                        QUEUES3[(wi + s) % 3].dma_start(
                            wt[psl, g, :], wd[psl, g, :])

            def emit_wo():
                for h in range(HPC):
                    (nc.scalar, nc.gpsimd)[h].dma_start(
                        wo_t[:, h, :], wob[:, h, :])

            def emit_pair_dmas(pair, at_c=None):
                xts = []
                for c in range(CCH):
                    xt = xp.tile([P, 2, 512], bf16, tag="x",
                                 name=f"x_{pair}_{c}")
                    if pair == 0 and c == 0:
                        # 4-way partition split across queues for startup latency
                        for s in range(4):
                            psl = slice(s * 32, (s + 1) * 32)
                            QUEUES3[s % 3].dma_start(
                                xt[psl, :, :], xb[pair, c, psl])
                    elif pair == 0:
                        # 2-way split, keeping all queues fed
                        for s in range(2):
                            psl = slice(s * 64, (s + 1) * 64)
                            QUEUES3[(c + 2 * s) % 3].dma_start(
                                xt[psl, :, :], xb[pair, c, psl])
                    else:
                        nc.sync.dma_start(xt[:], xb[pair, c])
                    xts.append(xt)
                    if at_c is not None and c in at_c:
                        at_c[c]()
                cos_t = csp.tile([P, 2, 512], bf16, tag="cos", name=f"cos{pair}")
                nc.scalar.dma_start(cos_t[:], csb[pair])
                sin_t = csp.tile([P, 2, 512], bf16, tag="sin", name=f"sin{pair}")
                nc.gpsimd.dma_start(sin_t[:], snb[pair])
                return xts, cos_t, sin_t

            # ---- per-slot projection state ----
            def alloc_proj(tt):
                pq = ps.tile([P, 2, 512], f32, tag="qk", bufs=2, name=f"pq{tt}")
                pk = ps.tile([P, 2, 512], f32, tag="qk", bufs=2, name=f"pk{tt}")
                pv0 = ps.tile([P, 2, 256], f32, tag="pv", bufs=2, name=f"pv0_{tt}")
                pv1 = ps.tile([P, 2, 256], f32, tag="pv", bufs=2, name=f"pv1_{tt}")
                return pq, pk, pv0, pv1

            def emit_proj_c(tt, c, xts, pq, pk, pv0, pv1):
                half = tt % 2
                xt = xts[c]
                xtr = xt[:, half, :]
                g, ci = c // 4, c % 4
                st, sp = (c == 0), (c == CCH - 1)
                tsl = slice(tt * 512, (tt + 1) * 512)
                for h in range(HPC):
                    wsl = slice(ci * 256 + h * 128, ci * 256 + (h + 1) * 128)
                    nc.tensor.matmul(pq[:, h, :], wq_t[:, g, wsl], xtr,
                                     start=st, stop=sp,
                                     skip_group_check=(h == 1))
                    nc.tensor.matmul(pk[:, h, :], wk_t[:, g, wsl], xtr,
                                     start=st, stop=sp,
                                     skip_group_check=(h == 1))
                if sp:
                    # q/k stopped: start their drains now, overlapped with
                    # c15's v matmuls, so phase B's first py never waits
                    nc.scalar.copy(qT_t[:, 0:2, tsl], pq[:, :, :])
                    nc.vector.tensor_copy(kT_t[:, 0:2, tsl], pk[:, :, :])
                vr = wv_t[:, g, ci * 256:(ci + 1) * 256]
                for s4 in range(4):
                    pvt = pv0 if s4 < 2 else pv1
                    nc.tensor.matmul(pvt[:, s4 % 2, :],
                                     xt[:, half, s4 * 128:(s4 + 1) * 128], vr,
                                     start=st and (s4 % 2 == 0), stop=sp,
                                     skip_group_check=(s4 % 2 == 1))

            def emit_drains(tt, pq, pk, pv0, pv1):
                # q/k drains were emitted inline at c15 (see emit_proj_c)
                nc.scalar.copy(v_t[:, tt * 4:tt * 4 + 2, :], pv0[:, :, :])
                nc.scalar.copy(v_t[:, tt * 4 + 2:tt * 4 + 4, :], pv1[:, :, :])

            def emit_rope(tt, cos_t, sin_t):
                half = tt % 2
                tsl = slice(tt * 512, (tt + 1) * 512)
                for ti, dst_t in enumerate((qT_t, kT_t)):
                    eng = nc.vector
                    for h in range(HPC):
                        dst = dst_t[:, h, tsl]
                        rot = ropep.tile([P, 512], bf16, tag=f"rot{ti}",
                                         name=f"rot{tt}{h}")
                        eng.tensor_scalar_mul(rot[0:64, :],
                                              dst[64:128, :], -1.0)
                        eng.tensor_copy(rot[64:128, :], dst[0:64, :])
                        eng.tensor_mul(out=rot[:], in0=rot[:],
                                       in1=sin_t[:, half, :])
                        eng.tensor_mul(out=dst, in0=dst,
                                       in1=cos_t[:, half, :])
                        eng.tensor_add(out=dst, in0=dst, in1=rot[:])

            # ---- attention unit pieces ----
            pt_store = {}
            acc_store = {}
            onorm_store = {}

            def score_slots_for(u):
                qt = u % 4
                return [(h, pi) for h in range(HPC) for pi in range(2 * (qt + 1))]

            def emit_scores(u, h, pi, ptag="pp"):
                b, qt = u // 4, u % 4
                qsl = slice(b * T + qt * 512, b * T + qt * 512 + 512)
                qr = qT_t[:, h, qsl]
                pp = ps.tile([P, 2, 512], f32, tag=ptag,
                             bufs=(1 if ptag == "pp" else 2),
                             name=f"pp{u}{h}{pi}")
                for j in (0, 1):
                    kt = 2 * pi + j
                    off = max(0, (kt - 4 * qt)) * P
                    ksl = slice(b * T + kt * P, b * T + (kt + 1) * P)
                    nc.tensor.matmul(pp[:, j, off:512],
                                     kT_t[:, h, ksl], qr[:, off:512],
                                     start=True, stop=True,
                                     skip_group_check=(j == 1))
                pt = ptp.tile([P, 2, 512], bf16, tag="pt", name=f"pt{u}{h}{pi}")
                dp = pi - 2 * qt
                if dp == 1:
                    # leading cols fully masked on both kt's: skip them in exp
                    nc.scalar.activation(pt[:, 0, 256:512], pp[:, 0, 256:512],
                                         Exp, scale=inv_sqrt_hd)
                    nc.scalar.activation(pt[:, 1, 384:512], pp[:, 1, 384:512],
                                         Exp, scale=inv_sqrt_hd)
                else:
                    nc.scalar.activation(pt[:], pp[:], Exp, scale=inv_sqrt_hd)
                if 0 <= dp < 2:
                    # triangular mask on the diagonal 128-col band only
                    for j in (0, 1):
                        jj = 2 * dp + j
                        band = slice(jj * 128, (jj + 1) * 128)
                        nc.vector.tensor_mul(out=pt[:, j, band],
                                             in0=pt[:, j, band], in1=tri[:])
                # row-sum accumulation (over kt tiles; per-column offsets skip
                # the never-written cols of diagonal tiles)
                if pi == 0:
                    acc = accp.tile([P, 512], f16, tag="acc", name=f"acc{u}{h}")
                    acc_store[(u, h)] = acc
                    if dp == 0:  # qt == 0: first pair is diagonal
                        nc.vector.tensor_copy(acc[:], pt[:, 0, :])
                        nc.vector.tensor_add(out=acc[:, 128:512],
                                             in0=acc[:, 128:512],
                                             in1=pt[:, 1, 128:512])
                    else:
                        nc.vector.tensor_add(out=acc[:], in0=pt[:, 0, :],
                                             in1=pt[:, 1, :])
                else:
                    acc = acc_store[(u, h)]
                    for j in (0, 1):
                        off = max(0, (2 * pi + j) - 4 * qt) * P
                        nc.vector.tensor_add(out=acc[:, off:512],
                                             in0=acc[:, off:512],
                                             in1=pt[:, j, off:512])
                pt_store[(u, h, pi)] = pt

            def emit_av_pair(u, h, pi, po):
                b, qt = u // 4, u % 4
                nkt = 4 * (qt + 1)
                pt = pt_store.pop((u, h, pi))
                for j in (0, 1):
                    kt = 2 * pi + j
                    off = max(0, (kt - 4 * qt)) * P
                    nc.tensor.matmul(po[:, off:512],
                                     v_t[:, b * (T // P) + kt,
                                         h * HD:(h + 1) * HD],
                                     pt[:, j, off:512],
                                     start=(kt == 0), stop=(kt == nkt - 1),
                                     skip_group_check=(off > 0))

            def alloc_po(u, h):
                return ps.tile([P, 512], f32, tag="pv", bufs=2, name=f"po{u}{h}")

            # --- softmax denominator path (per unit): one N=512 matmul per
            # head with ones stationary sums acc over key-partitions into a
            # [1,512] PSUM row; ACT copies both rows to SBUF bf16; one
            # broadcast matmul per head (ones_row stationary, LS=1) replicates
            # the row across 128 partitions; reciprocal_approx_fast + one DVE
            # mul normalize po into onorm. No transposes, no gpsimd.
            def emit_rsum(u, h, rsum2):
                # PE out base partition must be 0/32/64: head h row -> h*32
                acc = acc_store.pop((u, h))
                nc.tensor.matmul(rsum2[h * 32:h * 32 + 1, :], ones_col[:],
                                 acc[:], start=True, stop=True,
                                 skip_group_check=(h == 1))

            def emit_rcopy(u, rsum2):
                r_sb = rbcp.tile([64, 512], bf16, tag="rsb", name=f"rsb{u}")
                for h in range(HPC):
                    nc.scalar.copy(r_sb[h * 32:h * 32 + 1, :],
                                   rsum2[h * 32:h * 32 + 1, :])
                return r_sb

            def emit_bc(u, r_sb):
                bc2 = ps.tile([P, 2, 512], f32, tag="pp", bufs=1, name=f"bc{u}")
                for h in range(HPC):
                    # lhsT and rhs must share base partition (0 or 32)
                    nc.tensor.matmul(bc2[:, h, :],
                                     ones_rows[h * 32:h * 32 + 1, :],
                                     r_sb[h * 32:h * 32 + 1, :],
                                     start=True, stop=True,
                                     skip_group_check=(h == 1))
                return bc2

            def emit_norm_mul(u, h, bc2, po, onorm):
                rinv = rinvp.tile([P, 512], f32, tag="rinv", name=f"ri{u}{h}")
                nc.vector.reciprocal_approx_fast(rinv[:], bc2[:, h, :])
                nc.vector.tensor_mul(out=onorm[:, h, :], in0=po[:], in1=rinv[:])

            def emit_yproj_s4(w, s4, onorm, ystage):
                b, qt = w // 4, w % 4
                r0 = b * T + qt * 512 + s4 * P
                for dpair in range(2):
                    py = ps.tile([P, 2, 512], f32, tag="qk", bufs=2,
                                 name=f"py{w}{s4}{dpair}")
                    for d2 in range(2):
                        dout = dpair * 2 + d2
                        for h in range(HPC):
                            nc.tensor.matmul(
                                py[:, d2, :],
                                onorm[:, h, s4 * P:(s4 + 1) * P],
                                wo_t[:, h, dout * 512:(dout + 1) * 512],
                                start=(h == 0), stop=(h == HPC - 1),
                                skip_group_check=(d2 == 1))
                    # half-width drains, both on ACT: DVE runs RoPE early in
                    # phase B (see below) and must not gate the py rotation
                    nc.scalar.copy(ystage[:, 2 * dpair, :], py[:, 0, :])
                    nc.scalar.copy(ystage[:, 2 * dpair + 1, :], py[:, 1, :])
                nc.sync.dma_start(y[r0:r0 + P, 0:2, :], ystage[:, 0:2, :])
                nc.scalar.dma_start(y[r0:r0 + P, 2:4, :], ystage[:, 2:4, :])

            def emit_yproj(w, onorm):
                for s4 in range(4):
                    ystage = ysp.tile([P, 4, 512], f16, tag="ystage",
                                      name=f"ys{w}{s4}")
                    emit_yproj_s4(w, s4, onorm, ystage)

            # ================= main schedule =================
            cur_cos = cur_sin = None
            for tt in range(2 * NPAIR):
                pair, half = tt // 2, tt % 2
                if half == 0:
                    at_c = None
                    if pair == 0:
                        def late_groups():
                            emit_w_group(2, 2)
                            emit_w_group(3, 2)
                            emit_wo()
                        at_c = {
                            3: lambda: emit_w_group(1, 2),
                            15: late_groups,
                        }
                    xts, cur_cos, cur_sin = emit_pair_dmas(pair, at_c)
                    cur_xts = xts
                    if pair == 0:
                        emit_consts()
                u = tt - 1
                w = tt - 2
                sslots = score_slots_for(u) if u >= 0 else []
                # scores one-per-c-iter, ending 2 c-iters before phase B so
                # the last exps+acc-adds drain off ACT/DVE inside phase A
                off_c = max(0, CCH - len(sslots) - 2)
                pq, pk, pv0, pv1 = alloc_proj(tt)
                for c in range(CCH):
                    emit_proj_c(tt, c, cur_xts, pq, pk, pv0, pv1)
                    si = c - off_c
                    if 0 <= si < len(sslots):
                        emit_scores(u, *sslots[si])
                # ---- phase B ----
                emit_drains(tt, pq, pk, pv0, pv1)
                # RoPE immediately: q/k drains completed at c15, and putting
                # it early in the DVE queue means it finishes inside this
                # slot instead of 3us into the next one
                emit_rope(tt, cur_cos, cur_sin)
                if u >= 0:
                    onorm = onp.tile([P, HPC, 512], bf16, tag="onorm",
                                     name=f"on{u}")
                    onorm_store[u] = onorm
                    rsum2 = ps.tile([64, 512], f32, tag="pp", bufs=1,
                                    name=f"rs{u}")
                    po_h = []
                    for h in range(HPC):
                        po = alloc_po(u, h)
                        for pi in range(2 * (u % 4 + 1)):
                            emit_av_pair(u, h, pi, po)
                        po_h.append(po)
                        emit_rsum(u, h, rsum2)
                    r_sb = emit_rcopy(u, rsum2)
                if w >= 0:
                    emit_yproj(w, onorm_store.pop(w))
                if u >= 0:
                    bc2 = emit_bc(u, r_sb)
                    for h in range(HPC):
                        emit_norm_mul(u, h, bc2, po_h[h], onorm)

            # ================= tail: unit 7 + yproj(6) + yproj(7) ==========
            u = 2 * NPAIR - 1  # 7
            w = u - 1          # 6
            onorm_store[u] = onp.tile([P, HPC, 512], bf16, tag="onorm",
                                      name=f"on{u}")
            on_w = onorm_store.pop(w)
            sslots = score_slots_for(u)  # 16, head-major
            ystages = [ysp.tile([P, 4, 512], f16, tag="ystage", name=f"ys{w}{s}")
                       for s in range(4)]
            po0 = None
            for s, (h, pi) in enumerate(sslots):
                # alternate PSUM tags: qk's 2 bufs + pp's 1 give ~3-deep
                # score lookahead so the tail never paces at exp latency
                emit_scores(u, h, pi, ptag=("pp" if s % 2 == 0 else "qk"))
                if s < 4:
                    emit_yproj_s4(w, s, on_w, ystages[s])
                if s >= 8:
                    if po0 is None:
                        po0 = alloc_po(u, 0)
                    emit_av_pair(u, 0, s - 8, po0)
            on_u = onorm_store.pop(u)
            rsum2 = ps.tile([64, 512], f32, tag="pp", bufs=1, name=f"rs{u}")
            emit_rsum(u, 0, rsum2)
            po1 = alloc_po(u, 1)
            for pi in range(8):
                emit_av_pair(u, 1, pi, po1)
            emit_rsum(u, 1, rsum2)
            r_sb = emit_rcopy(u, rsum2)
            bc2 = emit_bc(u, r_sb)
            emit_norm_mul(u, 0, bc2, po0, on_u)
            emit_norm_mul(u, 1, bc2, po1, on_u)
            emit_yproj(u, on_u)

    nc.compile()
    return nc


def get_nc():
    if "nc" not in _CACHE:
        _CACHE["nc"] = _build_nc()
    return _CACHE["nc"]


def make_in_maps(x, cos, sin, wq, wk, wv, wo):
    bf16 = ml_dtypes.bfloat16
    xT = np.ascontiguousarray(x.reshape(TOK, D).T).astype(bf16)  # [D, TOK]
    # xb[pair, c, p, half, j] = xT[c*128+p, pair*1024 + half*512 + j]
    xb = np.ascontiguousarray(
        xT.reshape(CCH, P, NPAIR, 2, 512).transpose(2, 0, 1, 3, 4))
    cosT = np.ascontiguousarray(cos.reshape(TOK, HD).T).astype(bf16)
    sinT = np.ascontiguousarray(sin.reshape(TOK, HD).T).astype(bf16)
    csb = np.ascontiguousarray(cosT.reshape(HD, NPAIR, 2, 512).transpose(1, 0, 2, 3))
    snb = np.ascontiguousarray(sinT.reshape(HD, NPAIR, 2, 512).transpose(1, 0, 2, 3))
    in_maps = []
    for c in range(NCORES):
        dsl = slice(c * DC, (c + 1) * DC)

        def wimg(w):
            # [D, DC] -> [p, g, ci*256+dd] with contraction k = (4g+ci)*128+p
            wT = np.ascontiguousarray(w[dsl, :].T).astype(bf16)
            return np.ascontiguousarray(
                wT.reshape(4, 4, P, DC).transpose(2, 0, 1, 3).reshape(P, 4, 1024))

        woT = np.ascontiguousarray(wo[:, dsl].T).astype(bf16)  # [DC, D]
        wob = np.ascontiguousarray(woT.reshape(HPC, P, D).transpose(1, 0, 2))
        in_maps.append({
            "xb": xb, "csb": csb, "snb": snb,
            "wqb": wimg(wq), "wkb": wimg(wk), "wvb": wimg(wv),
            "wob": wob,
        })
    return in_maps


def kernel(x, cos, sin, wq, wk, wv, wo):
    from concourse.bass_utils import run_bass_kernel_spmd

    nc = get_nc()
    in_maps = make_in_maps(
        np.asarray(x, dtype=np.float32), np.asarray(cos, dtype=np.float32),
        np.asarray(sin, dtype=np.float32), np.asarray(wq, dtype=np.float32),
        np.asarray(wk, dtype=np.float32), np.asarray(wv, dtype=np.float32),
        np.asarray(wo, dtype=np.float32))
    res = run_bass_kernel_spmd(nc, in_maps, list(range(NCORES)))
    out = np.zeros((TOK, D), dtype=np.float32)
    for m in res.results:
        out += m["y"].reshape(TOK, D).astype(np.float32)
    return out.reshape(B, T, D)


# revision 26
# speedup vs baseline: 1.0211x; 1.0211x over previous
"""Trainium2 Bass kernel for causal multi-head attention with RoPE.

Problem: x[2,2048,2048], 16 heads, head_dim 128, fp32.
  q/k/v = x @ w{q,k,v}^T ; RoPE on q,k ; causal softmax(q k^T / sqrt(128)) @ v ; out @ wo^T

Sharding: Megatron tensor-parallel over heads - 2 heads per core on 8 cores.
Each core computes a partial y (its 2 heads' contribution through wo); the host
sums the 8 partials.  No device collectives.

Schedule (v2, restructured from the 395us baseline after trace analysis showed
PE stalling ~1us per kt-pair on the ACT exp latency inside attention units):
  - each half-tile slot tt = [phase A: projection c-loop x16 with unit tt-1's
    score matmuls+exp interleaved between c-iterations] then [phase B: drains,
    yproj(tt-2), AV(tt-1), normalize, RoPE(tt)].  Scores are emitted as late
    as possible in phase A (offset CCH - nslots) while still giving every exp
    a full c-iteration of PE work to hide under; by AV time all exps are done,
    so phase B's AV+yproj matmuls run back-to-back.
  - probs pt live in SBUF (16 tiles), decoupling exp from AV; PSUM is
    statically 8 banks: qk(2x2, reused by yproj py), pv(2x1, reused by AV po),
    pp(1x2 for in-flight scores).
  - softmax denominators: DVE pair-sum accumulation into f16 acc as before,
    then ONE gpsimd partition_all_reduce per head (replaces the rowsum
    matmuls + PE transposes + ACT copy + broadcast chain), DVE reciprocal
    [128,512] -> bf16, one DVE mul onorm = po * rinv.
  - causal masking: single [128,128] triangular mask applied to the diagonal
    128-col band only (cols below the band are excluded from exp/AV/acc by
    offsets); exp split on the second diagonal pair to skip fully-masked cols.
  - drains: q/k PSUM->SBUF casts on gpsimd (idle engine), v+ystage upper on
    ACT, ystage lower on DVE.
  - startup: x chunk0 + weight group-0 DMAs 4-way partition-split round-robin
    over 4 queues (sync/gpsimd/scalar/vector) before any const emission, so
    the first c-loop starts ~8us earlier.
"""

import math
import sys

sys.path.insert(0, "/opt/trn_rl_repo")

import ml_dtypes  # noqa: E402
import numpy as np  # noqa: E402

P = 128
D = 2048
HD = 128  # head dim
B = 2
T = 2048
TOK = B * T  # 4096
NCORES = 8
HPC = 2  # heads per core
DC = HPC * HD  # 256 dims per core
CCH = D // P  # 16 contraction chunks
NPAIR = TOK // 1024  # 4 token-tile pairs (1024 tokens each)
QT = T // 512  # 4 query tiles per batch

_CACHE = {}


def _build_nc():
    import concourse.bacc as bacc

    import concourse.mybir as mybir
    import concourse.tile as tile

    f32 = mybir.dt.float32
    bf16 = mybir.dt.bfloat16
    f16 = mybir.dt.float16
    Exp = mybir.ActivationFunctionType.Exp

    nc = bacc.Bacc("TRN2", target_bir_lowering=False, debug=False, num_devices=NCORES)

    xb = nc.dram_tensor("xb", [NPAIR, CCH, P, 2, 512], bf16, kind="ExternalInput").ap()
    csb = nc.dram_tensor("csb", [NPAIR, P, 2, 512], bf16, kind="ExternalInput").ap()
    snb = nc.dram_tensor("snb", [NPAIR, P, 2, 512], bf16, kind="ExternalInput").ap()
    wqb = nc.dram_tensor("wqb", [P, 4, 1024], bf16, kind="ExternalInput").ap()
    wkb = nc.dram_tensor("wkb", [P, 4, 1024], bf16, kind="ExternalInput").ap()
    wvb = nc.dram_tensor("wvb", [P, 4, 1024], bf16, kind="ExternalInput").ap()
    wob = nc.dram_tensor("wob", [P, HPC, D], bf16, kind="ExternalInput").ap()
    y = nc.dram_tensor("y", [TOK, 4, 512], f16, kind="ExternalOutput").ap()

    inv_sqrt_hd = 1.0 / math.sqrt(HD)

    with tile.TileContext(nc) as tc:
        with (
            tc.tile_pool(name="consts", bufs=1) as consts,
            tc.tile_pool(name="wpool", bufs=1) as wpool,
            tc.tile_pool(name="qkv", bufs=1) as qkv,
            tc.tile_pool(name="xp", bufs=17) as xp,
            tc.tile_pool(name="csp", bufs=2) as csp,
            tc.tile_pool(name="ropep", bufs=2) as ropep,
            tc.tile_pool(name="ptp", bufs=16) as ptp,
            tc.tile_pool(name="accp", bufs=4) as accp,
            tc.tile_pool(name="rbcp", bufs=2) as rbcp,
            tc.tile_pool(name="rinvp", bufs=2) as rinvp,
            tc.tile_pool(name="onp", bufs=3) as onp,
            tc.tile_pool(name="ysp", bufs=3) as ysp,
            tc.tile_pool(name="ps", bufs=1, space="PSUM") as ps,
        ):
            # ---- resident weights (SBUF-image dram layouts: 2KB+ rows) ----
            wq_t = wpool.tile([P, 4, 1024], bf16, tag="wq", name="wq_t")
            wk_t = wpool.tile([P, 4, 1024], bf16, tag="wk", name="wk_t")
            wv_t = wpool.tile([P, 4, 1024], bf16, tag="wv", name="wv_t")
            wo_t = wpool.tile([P, HPC, D], bf16, tag="wo", name="wo_t")

            # ---- resident activations ----
            qT_t = qkv.tile([P, HPC, TOK], bf16, tag="qT", name="qT_t")
            kT_t = qkv.tile([P, HPC, TOK], bf16, tag="kT", name="kT_t")
            v_t = qkv.tile([P, TOK // P, DC], bf16, tag="v", name="v_t")

            QUEUES3 = [nc.sync, nc.gpsimd, nc.scalar]

            # ---- startup burst (baseline pattern): weight group-0 4-way
            # partition-split across the 3 DMA queues, before anything else.
            for wi, (wt, wd) in enumerate(((wq_t, wqb), (wk_t, wkb), (wv_t, wvb))):
                for s4 in range(4):
                    psl = slice(s4 * 32, (s4 + 1) * 32)
                    QUEUES3[(wi + s4) % 3].dma_start(wt[psl, 0, :], wd[psl, 0, :])

            # ---- constants (tiny; tri not needed until slot 1's scores) ----
            # triangular 0/1 mask for the diagonal 128-col band: keep c >= p
            tri = consts.tile([P, P], bf16, tag="tri", name="tri")
            ones_col = consts.tile([P, 1], f16, tag="ones_col", name="ones_col")
            ones_rows = consts.tile([64, P], bf16, tag="ones_rows", name="ones_rows")

            dummy = consts.tile([P, 512], bf16, tag="dummy", name="dummy")
            nc.vector.memset(dummy[:], 0.0)
            pdum = ps.tile([P, 2, 512], f32, tag="pp", bufs=1, name="pdum")
            for _ in range(14):
                # p-state warm-up: keep the PE streaming while the startup
                # DMAs land, so the first real c-iters run at full clock
                nc.tensor.matmul(pdum[:, 0, :], dummy[:, 0:128], dummy[:],
                                 start=True, stop=True, skip_group_check=True)

            def emit_consts():
                nc.gpsimd.memset(tri[:], 1.0)
                nc.gpsimd.affine_select(
                    out=tri[:], in_=tri[:], compare_op=mybir.AluOpType.is_ge,
                    fill=0.0, base=0, channel_multiplier=-1, pattern=[[1, P]],
                )
                nc.gpsimd.memset(ones_col[:], 1.0)
                nc.gpsimd.memset(ones_rows[:], 1.0)

            def emit_w_group(g, nsplit):
                for wi, (wt, wd) in enumerate(((wq_t, wqb), (wk_t, wkb),
                                               (wv_t, wvb))):
                    for s in range(nsplit):
                        psl = slice(s * (P // nsplit), (s + 1) * (P // nsplit))
                        QUEUES3[(wi + s) % 3].dma_start(
                            wt[psl, g, :], wd[psl, g, :])

            def emit_wo():
                for h in range(HPC):
                    (nc.scalar, nc.gpsimd)[h].dma_start(
                        wo_t[:, h, :], wob[:, h, :])

            def emit_pair_dmas(pair, at_c=None):
                xts = []
                for c in range(CCH):
                    xt = xp.tile([P, 2, 512], bf16, tag="x",
                                 name=f"x_{pair}_{c}")
                    if pair == 0 and c == 0:
                        # 4-way partition split across queues for startup latency
                        for s in range(4):
                            psl = slice(s * 32, (s + 1) * 32)
                            QUEUES3[s % 3].dma_start(
                                xt[psl, :, :], xb[pair, c, psl])
                    elif pair == 0:
                        # 2-way split, keeping all queues fed
                        for s in range(2):
                            psl = slice(s * 64, (s + 1) * 64)
                            QUEUES3[(c + 2 * s) % 3].dma_start(
                                xt[psl, :, :], xb[pair, c, psl])
                    else:
                        nc.sync.dma_start(xt[:], xb[pair, c])
                    xts.append(xt)
                    if at_c is not None and c in at_c:
                        at_c[c]()
                cos_t = csp.tile([P, 2, 512], bf16, tag="cos", name=f"cos{pair}")
                nc.scalar.dma_start(cos_t[:], csb[pair])
                sin_t = csp.tile([P, 2, 512], bf16, tag="sin", name=f"sin{pair}")
                nc.gpsimd.dma_start(sin_t[:], snb[pair])
                return xts, cos_t, sin_t

            # ---- per-slot projection state ----
            def alloc_proj(tt):
                pq = ps.tile([P, 2, 512], f32, tag="qk", bufs=2, name=f"pq{tt}")
                pk = ps.tile([P, 2, 512], f32, tag="qk", bufs=2, name=f"pk{tt}")
                pv0 = ps.tile([P, 2, 256], f32, tag="pv", bufs=2, name=f"pv0_{tt}")
                pv1 = ps.tile([P, 2, 256], f32, tag="pv", bufs=2, name=f"pv1_{tt}")
                return pq, pk, pv0, pv1

            def emit_proj_c(tt, c, xts, pq, pk, pv0, pv1):
                half = tt % 2
                xt = xts[c]
                xtr = xt[:, half, :]
                g, ci = c // 4, c % 4
                st, sp = (c == 0), (c == CCH - 1)
                tsl = slice(tt * 512, (tt + 1) * 512)
                for h in range(HPC):
                    wsl = slice(ci * 256 + h * 128, ci * 256 + (h + 1) * 128)
                    nc.tensor.matmul(pq[:, h, :], wq_t[:, g, wsl], xtr,
                                     start=st, stop=sp,
                                     skip_group_check=(h == 1))
                    nc.tensor.matmul(pk[:, h, :], wk_t[:, g, wsl], xtr,
                                     start=st, stop=sp,
                                     skip_group_check=(h == 1))
                vr = wv_t[:, g, ci * 256:(ci + 1) * 256]
                for s4 in range(4):
                    pvt = pv0 if s4 < 2 else pv1
                    nc.tensor.matmul(pvt[:, s4 % 2, :],
                                     xt[:, half, s4 * 128:(s4 + 1) * 128], vr,
                                     start=st and (s4 % 2 == 0), stop=sp,
                                     skip_group_check=(s4 % 2 == 1))

            def emit_drains(tt, pq, pk, pv0, pv1):
                tsl = slice(tt * 512, (tt + 1) * 512)
                nc.scalar.copy(qT_t[:, 0:2, tsl], pq[:, :, :])
                nc.vector.tensor_copy(kT_t[:, 0:2, tsl], pk[:, :, :])
                nc.scalar.copy(v_t[:, tt * 4:tt * 4 + 2, :], pv0[:, :, :])
                nc.scalar.copy(v_t[:, tt * 4 + 2:tt * 4 + 4, :], pv1[:, :, :])

            def emit_rope(tt, cos_t, sin_t):
                half = tt % 2
                tsl = slice(tt * 512, (tt + 1) * 512)
                for ti, dst_t in enumerate((qT_t, kT_t)):
                    eng = nc.vector
                    for h in range(HPC):
                        dst = dst_t[:, h, tsl]
                        rot = ropep.tile([P, 512], bf16, tag=f"rot{ti}",
                                         name=f"rot{tt}{h}")
                        eng.tensor_scalar_mul(rot[0:64, :],
                                              dst[64:128, :], -1.0)
                        eng.tensor_copy(rot[64:128, :], dst[0:64, :])
                        eng.tensor_mul(out=rot[:], in0=rot[:],
                                       in1=sin_t[:, half, :])
                        eng.tensor_mul(out=dst, in0=dst,
                                       in1=cos_t[:, half, :])
                        eng.tensor_add(out=dst, in0=dst, in1=rot[:])

            # ---- attention unit pieces ----
            pt_store = {}
            acc_store = {}
            onorm_store = {}

            def score_slots_for(u):
                qt = u % 4
                return [(h, pi) for h in range(HPC) for pi in range(2 * (qt + 1))]

            def emit_scores(u, h, pi, ptag="pp"):
                b, qt = u // 4, u % 4
                qsl = slice(b * T + qt * 512, b * T + qt * 512 + 512)
                qr = qT_t[:, h, qsl]
                pp = ps.tile([P, 2, 512], f32, tag=ptag,
                             bufs=(1 if ptag == "pp" else 2),
                             name=f"pp{u}{h}{pi}")
                for j in (0, 1):
                    kt = 2 * pi + j
                    off = max(0, (kt - 4 * qt)) * P
                    ksl = slice(b * T + kt * P, b * T + (kt + 1) * P)
                    nc.tensor.matmul(pp[:, j, off:512],
                                     kT_t[:, h, ksl], qr[:, off:512],
                                     start=True, stop=True,
                                     skip_group_check=(j == 1))
                pt = ptp.tile([P, 2, 512], bf16, tag="pt", name=f"pt{u}{h}{pi}")
                dp = pi - 2 * qt
                if dp == 1:
                    # leading cols fully masked on both kt's: skip them in exp
                    nc.scalar.activation(pt[:, 0, 256:512], pp[:, 0, 256:512],
                                         Exp, scale=inv_sqrt_hd)
                    nc.scalar.activation(pt[:, 1, 384:512], pp[:, 1, 384:512],
                                         Exp, scale=inv_sqrt_hd)
                else:
                    nc.scalar.activation(pt[:], pp[:], Exp, scale=inv_sqrt_hd)
                if 0 <= dp < 2:
                    # triangular mask on the diagonal 128-col band only
                    for j in (0, 1):
                        jj = 2 * dp + j
                        band = slice(jj * 128, (jj + 1) * 128)
                        nc.vector.tensor_mul(out=pt[:, j, band],
                                             in0=pt[:, j, band], in1=tri[:])
                # row-sum accumulation (over kt tiles; per-column offsets skip
                # the never-written cols of diagonal tiles)
                if pi == 0:
                    acc = accp.tile([P, 512], f16, tag="acc", name=f"acc{u}{h}")
                    acc_store[(u, h)] = acc
                    if dp == 0:  # qt == 0: first pair is diagonal
                        nc.vector.tensor_copy(acc[:], pt[:, 0, :])
                        nc.vector.tensor_add(out=acc[:, 128:512],
                                             in0=acc[:, 128:512],
                                             in1=pt[:, 1, 128:512])
                    else:
                        nc.vector.tensor_add(out=acc[:], in0=pt[:, 0, :],
                                             in1=pt[:, 1, :])
                else:
                    acc = acc_store[(u, h)]
                    for j in (0, 1):
                        off = max(0, (2 * pi + j) - 4 * qt) * P
                        nc.vector.tensor_add(out=acc[:, off:512],
                                             in0=acc[:, off:512],
                                             in1=pt[:, j, off:512])
                pt_store[(u, h, pi)] = pt

            def emit_av_pair(u, h, pi, po):
                b, qt = u // 4, u % 4
                nkt = 4 * (qt + 1)
                pt = pt_store.pop((u, h, pi))
                for j in (0, 1):
                    kt = 2 * pi + j
                    off = max(0, (kt - 4 * qt)) * P
                    nc.tensor.matmul(po[:, off:512],
                                     v_t[:, b * (T // P) + kt,
                                         h * HD:(h + 1) * HD],
                                     pt[:, j, off:512],
                                     start=(kt == 0), stop=(kt == nkt - 1),
                                     skip_group_check=(off > 0))

            def alloc_po(u, h):
                return ps.tile([P, 512], f32, tag="pv", bufs=2, name=f"po{u}{h}")

            # --- softmax denominator path (per unit): one N=512 matmul per
            # head with ones stationary sums acc over key-partitions into a
            # [1,512] PSUM row; ACT copies both rows to SBUF bf16; one
            # broadcast matmul per head (ones_row stationary, LS=1) replicates
            # the row across 128 partitions; reciprocal_approx_fast + one DVE
            # mul normalize po into onorm. No transposes, no gpsimd.
            def emit_rsum(u, h, rsum2):
                # PE out base partition must be 0/32/64: head h row -> h*32
                acc = acc_store.pop((u, h))
                nc.tensor.matmul(rsum2[h * 32:h * 32 + 1, :], ones_col[:],
                                 acc[:], start=True, stop=True,
                                 skip_group_check=(h == 1))

            def emit_rcopy(u, rsum2):
                r_sb = rbcp.tile([64, 512], bf16, tag="rsb", name=f"rsb{u}")
                for h in range(HPC):
                    nc.scalar.copy(r_sb[h * 32:h * 32 + 1, :],
                                   rsum2[h * 32:h * 32 + 1, :])
                return r_sb

            def emit_bc(u, r_sb):
                bc2 = ps.tile([P, 2, 512], f32, tag="pp", bufs=1, name=f"bc{u}")
                for h in range(HPC):
                    # lhsT and rhs must share base partition (0 or 32)
                    nc.tensor.matmul(bc2[:, h, :],
                                     ones_rows[h * 32:h * 32 + 1, :],
                                     r_sb[h * 32:h * 32 + 1, :],
                                     start=True, stop=True,
                                     skip_group_check=(h == 1))
                return bc2

            def emit_norm_mul(u, h, bc2, po, onorm):
                rinv = rinvp.tile([P, 512], f32, tag="rinv", name=f"ri{u}{h}")
                nc.vector.reciprocal_approx_fast(rinv[:], bc2[:, h, :])
                nc.vector.tensor_mul(out=onorm[:, h, :], in0=po[:], in1=rinv[:])

            def emit_yproj_s4(w, s4, onorm, ystage):
                b, qt = w // 4, w % 4
                r0 = b * T + qt * 512 + s4 * P
                for dpair in range(2):
                    py = ps.tile([P, 2, 512], f32, tag="qk", bufs=2,
                                 name=f"py{w}{s4}{dpair}")
                    for d2 in range(2):
                        dout = dpair * 2 + d2
                        for h in range(HPC):
                            nc.tensor.matmul(
                                py[:, d2, :],
                                onorm[:, h, s4 * P:(s4 + 1) * P],
                                wo_t[:, h, dout * 512:(dout + 1) * 512],
                                start=(h == 0), stop=(h == HPC - 1),
                                skip_group_check=(d2 == 1))
                    # half-width drains on DVE+ACT in parallel: py frees in
                    # ~0.6us so the qk-tag rotation never stalls the PE
                    nc.vector.tensor_copy(ystage[:, 2 * dpair, :], py[:, 0, :])
                    nc.scalar.copy(ystage[:, 2 * dpair + 1, :], py[:, 1, :])
                nc.sync.dma_start(y[r0:r0 + P, 0:2, :], ystage[:, 0:2, :])
                nc.scalar.dma_start(y[r0:r0 + P, 2:4, :], ystage[:, 2:4, :])

            def emit_yproj(w, onorm):
                for s4 in range(4):
                    ystage = ysp.tile([P, 4, 512], f16, tag="ystage",
                                      name=f"ys{w}{s4}")
                    emit_yproj_s4(w, s4, onorm, ystage)

            # ================= main schedule =================
            cur_cos = cur_sin = None
            for tt in range(2 * NPAIR):
                pair, half = tt // 2, tt % 2
                if half == 0:
                    at_c = None
                    if pair == 0:
                        def late_groups():
                            emit_w_group(2, 2)
                            emit_w_group(3, 2)
                            emit_wo()
                        at_c = {
                            3: lambda: emit_w_group(1, 2),
                            15: late_groups,
                        }
                    xts, cur_cos, cur_sin = emit_pair_dmas(pair, at_c)
                    cur_xts = xts
                    if pair == 0:
                        emit_consts()
                u = tt - 1
                w = tt - 2
                sslots = score_slots_for(u) if u >= 0 else []
                # scores one-per-c-iter, ending 2 c-iters before phase B so
                # the last exps+acc-adds drain off ACT/DVE inside phase A
                off_c = max(0, CCH - len(sslots) - 2)
                pq, pk, pv0, pv1 = alloc_proj(tt)
                for c in range(CCH):
                    emit_proj_c(tt, c, cur_xts, pq, pk, pv0, pv1)
                    si = c - off_c
                    if 0 <= si < len(sslots):
                        emit_scores(u, *sslots[si])
                # ---- phase B ----
                emit_drains(tt, pq, pk, pv0, pv1)
                if u >= 0:
                    onorm = onp.tile([P, HPC, 512], bf16, tag="onorm",
                                     name=f"on{u}")
                    onorm_store[u] = onorm
                    rsum2 = ps.tile([64, 512], f32, tag="pp", bufs=1,
                                    name=f"rs{u}")
                    po_h = []
                    for h in range(HPC):
                        po = alloc_po(u, h)
                        for pi in range(2 * (u % 4 + 1)):
                            emit_av_pair(u, h, pi, po)
                        po_h.append(po)
                        emit_rsum(u, h, rsum2)
                    r_sb = emit_rcopy(u, rsum2)
                if w >= 0:
                    emit_yproj(w, onorm_store.pop(w))
                if u >= 0:
                    bc2 = emit_bc(u, r_sb)
                    for h in range(HPC):
                        emit_norm_mul(u, h, bc2, po_h[h], onorm)
                emit_rope(tt, cur_cos, cur_sin)

            # ================= tail: unit 7 + yproj(6) + yproj(7) ==========
            u = 2 * NPAIR - 1  # 7
            w = u - 1          # 6
            onorm_store[u] = onp.tile([P, HPC, 512], bf16, tag="onorm",
                                      name=f"on{u}")
            on_w = onorm_store.pop(w)
            sslots = score_slots_for(u)  # 16, head-major
            ystages = [ysp.tile([P, 4, 512], f16, tag="ystage", name=f"ys{w}{s}")
                       for s in range(4)]
            po0 = None
            for s, (h, pi) in enumerate(sslots):
                # alternate PSUM tags: qk's 2 bufs + pp's 1 give ~3-deep
                # score lookahead so the tail never paces at exp latency
                emit_scores(u, h, pi, ptag=("pp" if s % 2 == 0 else "qk"))
                if s < 4:
                    emit_yproj_s4(w, s, on_w, ystages[s])
                if s >= 8:
                    if po0 is None:
                        po0 = alloc_po(u, 0)
                    emit_av_pair(u, 0, s - 8, po0)
            on_u = onorm_store.pop(u)
            rsum2 = ps.tile([64, 512], f32, tag="pp", bufs=1, name=f"rs{u}")
            emit_rsum(u, 0, rsum2)
            po1 = alloc_po(u, 1)
            for pi in range(8):
                emit_av_pair(u, 1, pi, po1)
            emit_rsum(u, 1, rsum2)
            r_sb = emit_rcopy(u, rsum2)
            bc2 = emit_bc(u, r_sb)
            emit_norm_mul(u, 0, bc2, po0, on_u)
            emit_norm_mul(u, 1, bc2, po1, on_u)
            emit_yproj(u, on_u)

    nc.compile()
    return nc


def get_nc():
    if "nc" not in _CACHE:
        _CACHE["nc"] = _build_nc()
    return _CACHE["nc"]


def make_in_maps(x, cos, sin, wq, wk, wv, wo):
    bf16 = ml_dtypes.bfloat16
    xT = np.ascontiguousarray(x.reshape(TOK, D).T).astype(bf16)  # [D, TOK]
    # xb[pair, c, p, half, j] = xT[c*128+p, pair*1024 + half*512 + j]
    xb = np.ascontiguousarray(
        xT.reshape(CCH, P, NPAIR, 2, 512).transpose(2, 0, 1, 3, 4))
    cosT = np.ascontiguousarray(cos.reshape(TOK, HD).T).astype(bf16)
    sinT = np.ascontiguousarray(sin.reshape(TOK, HD).T).astype(bf16)
    csb = np.ascontiguousarray(cosT.reshape(HD, NPAIR, 2, 512).transpose(1, 0, 2, 3))
    snb = np.ascontiguousarray(sinT.reshape(HD, NPAIR, 2, 512).transpose(1, 0, 2, 3))
    in_maps = []
    for c in range(NCORES):
        dsl = slice(c * DC, (c + 1) * DC)

        def wimg(w):
            # [D, DC] -> [p, g, ci*256+dd] with contraction k = (4g+ci)*128+p
            wT = np.ascontiguousarray(w[dsl, :].T).astype(bf16)
            return np.ascontiguousarray(
                wT.reshape(4, 4, P, DC).transpose(2, 0, 1, 3).reshape(P, 4, 1024))

        woT = np.ascontiguousarray(wo[:, dsl].T).astype(bf16)  # [DC, D]
        wob = np.ascontiguousarray(woT.reshape(HPC, P, D).transpose(1, 0, 2))
        in_maps.append({
            "xb": xb, "csb": csb, "snb": snb,
            "wqb": wimg(wq), "wkb": wimg(wk), "wvb": wimg(wv),
            "wob": wob,
        })
    return in_maps


def kernel(x, cos, sin, wq, wk, wv, wo):
    from concourse.bass_utils import run_bass_kernel_spmd

    nc = get_nc()
    in_maps = make_in_maps(
        np.asarray(x, dtype=np.float32), np.asarray(cos, dtype=np.float32),
        np.asarray(sin, dtype=np.float32), np.asarray(wq, dtype=np.float32),
        np.asarray(wk, dtype=np.float32), np.asarray(wv, dtype=np.float32),
        np.asarray(wo, dtype=np.float32))
    res = run_bass_kernel_spmd(nc, in_maps, list(range(NCORES)))
    out = np.zeros((TOK, D), dtype=np.float32)
    for m in res.results:
        out += m["y"].reshape(TOK, D).astype(np.float32)
    return out.reshape(B, T, D)
